# revision 29
# baseline (speedup 1.0000x reference)
"""Aitchison-Aitken categorical kernel on 8 TRN2 NeuronCores.

Math (reference, NUM_LEVELS=4, n_feat=64):
    w_f     = log(1-h_f) - log(h_f/3)
    base    = sum_f log(h_f/3) - sum_f log(h_f) = -64*log(3)   (data independent)
    match   = sum_f w_f * 1[test_if == train_jf]           ([n_test, n_train])
    ld      = match + base
    out     = rowmax(ld) * exp(ld - rowmax(ld))
          ( = c * exp(ld) with c = ln(m)/m, m = e^{rowmax ld} )

Device algorithm (per core, data-parallel over test rows):
  - host pre-lays-out test/train as feature-major f16 with the 64 features
    duplicated to 128 partitions ([dup | dup]); the device then needs no
    transposes, no PSUM encode staging, and only ~300 DMA descriptors
  - level-indicator encodings A=[lvl0|lvl1], B=[lvl2|lvl3] via DVE
    tensor_scalar is_equal (w folded into the test side), K = 2 x 128
  - per m-tile (128 test rows): 4 psum blocks of [128,2048] f32 (4 banks,
    ring of 2), each = 4 matmuls of 1024 cols (A,A,B,B accumulate)
  - exp(match + base) via one ScalarE activation per 2048 block,
    PSUM -> bf16 ebuf (bf16: e^ld spans e^-157..e^+41)
  - row max via a running DVE tensor_tensor max fold (2x bf16), folded
    2048 -> 512 before the single 1x tensor_reduce
  - c = ln(m)/m; obuf = ebuf * c (DVE tensor_scalar 4x); DMA per 4096 cols
  - the finish chain (ln, cvec, mults, DMAs) of m-tile m is emitted
    between m+1's exps to keep the ACT queue bubble-free

Sharding: test_Xs rows across 8 cores; bandwidths/train_Xs replicated;
out [1024, 8192] f16 local per core, host-concatenated.
"""
import numpy as np
from contextlib import ExitStack

from concourse import bacc, hw_specs, mybir, tile
from concourse.bass_utils import run_bass_kernel_spmd

f32 = mybir.dt.float32
f16 = mybir.dt.float16
bf16 = mybir.dt.bfloat16
ACTF = mybir.ActivationFunctionType
ALU = mybir.AluOpType

N_CORES = 8
N_TEST, N_TRAIN, N_FEAT = 8192, 8192, 64
M_LOC = N_TEST // N_CORES          # 1024 test rows per core
P = 128                            # partitions
M_TILES = M_LOC // P               # 8
QCOL = 2048                        # psum block (4 banks f32)
NQ = 4                             # blocks per m-tile
NMM = 512                          # cols per matmul (one PSUM bank)
BASE = float(-N_FEAT * np.log(3.0))
NEG_INF = float(np.float32(-3.0e38))


# ---------------------------------------------------------------------------
# Activation-table patch: both Exp and Ln are needed; the stock fixpoint pass
# resolves each to the first table set containing it (exp_and_others vs
# natural_log), reloading the ACT tables (~1.3us) on every switch. Hide
# Exp/Ln from every other set so both resolve to natural_log_exp_and_others
# at its true act_func_set_id (list order/indices preserved).
_COMBINED = "natural_log_exp_and_others"
_orig_get_tables = hw_specs.get_activation_tables


def _patched_tables(module_arch):
    d = _orig_get_tables(module_arch)
    if _COMBINED not in d:
        return d
    hide = {ACTF.Exp, ACTF.Ln}
    return {k: (set(v) if k == _COMBINED else set(v) - hide)
            for k, v in d.items()}


hw_specs.get_activation_tables = _patched_tables
bacc.get_activation_tables = _patched_tables   # bacc imported its own ref
# ---------------------------------------------------------------------------


def _build():
    nc = bacc.Bacc(None, target_bir_lowering=False)
    bw_ext = nc.declare_dram_parameter("bandwidths", [N_FEAT], f32, isOutput=False)
    # host-prepped: feature-major f16, features duplicated across halves
    test_ext = nc.declare_dram_parameter("testT", [P, M_LOC], f16, isOutput=False)
    trainA_ext = nc.declare_dram_parameter("trainA", [P, N_TRAIN], f16, isOutput=False)
    trainB_ext = nc.declare_dram_parameter("trainB", [P, N_TRAIN], f16, isOutput=False)
    out_ext = nc.declare_dram_parameter("out", [M_LOC, N_TRAIN], f16, isOutput=True)

    with tile.TileContext(nc) as tc, ExitStack() as ctx:
        const = ctx.enter_context(tc.tile_pool(name="const", bufs=1))
        ps = ctx.enter_context(tc.tile_pool(name="ps", bufs=2, space="PSUM"))
        enc = ctx.enter_context(tc.tile_pool(name="enc", bufs=1))
        stats = ctx.enter_context(tc.tile_pool(name="stats", bufs=4))
        scr = ctx.enter_context(tc.tile_pool(name="scr", bufs=3))
        ebuf_pool = ctx.enter_context(tc.tile_pool(name="ebuf", bufs=3))
        out_pool = ctx.enter_context(tc.tile_pool(name="obuf", bufs=4))

        # ---- input DMAs first, all on the sync HWDGE (RTL descriptor gen;
        # the gpsimd SWDGE takes ~6us to software-generate 128 descriptors).
        # Chunk 0 and bandwidths lead since they gate the first matmul;
        # each train chunk is its own tile so its is_equal only waits for
        # its own DMA. ----------------------------------------------------
        NCH = 4
        CH = N_TRAIN // NCH
        sA = [enc.tile([P, CH], f16, name=f"sA{c}") for c in range(NCH)]
        sB = [enc.tile([P, CH], f16, name=f"sB{c}") for c in range(NCH)]
        nc.sync.dma_start(out=sA[0][:], in_=trainA_ext[:, 0:CH])
        bw = const.tile([N_FEAT, 1], f32)
        nc.sync.dma_start(out=bw[:], in_=bw_ext[:].rearrange("(f o) -> f o", o=1))
        nc.sync.dma_start(out=sB[0][:], in_=trainB_ext[:, 0:CH])
        dstT_t = enc.tile([P, M_LOC], f16)
        nc.sync.dma_start(out=dstT_t[:], in_=test_ext[:])
        for c in range(1, NCH):
            nc.sync.dma_start(out=sA[c][:], in_=trainA_ext[:, c * CH:(c + 1) * CH])
            nc.sync.dma_start(out=sB[c][:], in_=trainB_ext[:, c * CH:(c + 1) * CH])

        one_t = const.tile([N_FEAT, 1], f32)
        nc.vector.memset(one_t[:], 1.0)
        base_t = const.tile([P, 1], f32)
        nc.vector.memset(base_t[:], BASE)
        # dummy activation: triggers the combined Exp/Ln table load now
        scratch = const.tile([P, 1], f32)
        nc.scalar.activation(scratch[:], base_t[:], ACTF.Exp)
        lvlA = const.tile([P, 1], f32)
        nc.vector.memset(lvlA[0:64, :], 0.0)
        nc.vector.memset(lvlA[64:128, :], 1.0)
        lvlB = const.tile([P, 1], f32)
        nc.vector.memset(lvlB[0:64, :], 2.0)
        nc.vector.memset(lvlB[64:128, :], 3.0)

        # ---- w vector from bandwidths ----------------------------------
        lt = const.tile([N_FEAT, 1], f32)   # log(1 - h)
        nc.scalar.activation(lt[:], bw[:], ACTF.Ln, bias=one_t[:], scale=-1.0)
        lf = const.tile([N_FEAT, 1], f32)   # log(h/3)
        nc.scalar.activation(lf[:], bw[:], ACTF.Ln, scale=1.0 / 3.0)

        # ---- encodings: is_equal (DVE), w folded into test side.
        # Emission order tuned for the lead-in: train chunk0 A first (its
        # DMA lands before bw), then the w chain + test encodings (gating
        # the first matmuls), then the rest.
        tencA = enc.tile([P, M_LOC], f16)
        tencB = enc.tile([P, M_LOC], f16)
        w2 = const.tile([P, 1], f32)
        nc.vector.tensor_tensor(w2[0:64, :], lt[:], lf[:], op=ALU.subtract)
        nc.vector.tensor_copy(w2[64:128, :], w2[0:64, :])
        nc.vector.tensor_scalar(tencA[:], dstT_t[:], lvlA[:], w2[:],
                                op0=ALU.is_equal, op1=ALU.mult)
        nc.vector.tensor_scalar(tencB[:], dstT_t[:], lvlB[:], w2[:],
                                op0=ALU.is_equal, op1=ALU.mult)

        # ---- PE warm-up: the HAM clock gate starts matmuls at ~half rate
        # until ~4-5us of sustained activity; burn dummy matmuls into the
        # first psum tile while the input DMAs land so the real first
        # m-tile runs at full clock (start=True re-clears has_written).
        warm = const.tile([P, NMM], f16)
        nc.vector.memset(warm[:], 0.0)
        warm_pst = ps.tile([P, QCOL], f32, tag="mm")
        for i in range(10):
            nc.tensor.matmul(warm_pst[:, (i % 4) * NMM:(i % 4 + 1) * NMM],
                             warm[:, 0:P], warm[:], start=True, stop=True)

        # ---- main loop --------------------------------------------------
        # The finish chain of m-tile m (ln -> recip -> cvec -> mults ->
        # DMAs) is deferred and emitted between m+1's exp q1 and q2.  By
        # then m's DVE max-reduce is long done, so ACT reaches the Ln with
        # no wait and the queue never bubbles; m's output mults run on DVE
        # while ACT exps m+1's later blocks.
        pending = [None]

        def run_mtile(m):
            ms = slice(m * P, (m + 1) * P)
            last = m == M_TILES - 1
            ebuf = ebuf_pool.tile([P, N_TRAIN], bf16)
            mm_t = stats.tile([P, 1], f32, tag="mm_t", name="mm_t")
            # running pairwise-max fold over ebuf: gA = eb0 v eb1 (after
            # q1, runs during q2's exps), gB = gA v eb2, gC = gB v eb3,
            # then fold 2048 -> 512 and one 1x reduce (single-operand
            # reduce is 1x-only on DVE: keep its input short).
            gA = scr.tile([P, QCOL], bf16, tag="sc")
            gB = scr.tile([P, QCOL], bf16, tag="sc")
            gC = scr.tile([P, QCOL], bf16, tag="sc")
            if last and pending[0] is not None:
                # flush m-2's finish now so the DVE queue is clear for
                # this tile's fold ladder (its tail is the kernel tail);
                # costs at most a short ACT wait on ln once
                pending[0]()
                pending[0] = None

            for q in range(NQ):
                qs = slice(q * QCOL, (q + 1) * QCOL)
                pst = ps.tile([P, QCOL], f32, tag="mm")
                # A,A,B,B per block; alternate order across blocks so the
                # boundary stationary is shared (ldweights pull-ahead hides
                # the rest)
                order = ((tencA, sA[q]), (tencB, sB[q]))
                if q % 2 == 1:
                    order = (order[1], order[0])
                (t0, s0), (t1, s1) = order
                for j in range(QCOL // NMM):
                    js = slice(j * NMM, (j + 1) * NMM)
                    nc.tensor.matmul(pst[:, j * NMM:(j + 1) * NMM],
                                     t0[:, ms], s0[:, js],
                                     start=True, stop=False)
                for j in range(QCOL // NMM):
                    js = slice(j * NMM, (j + 1) * NMM)
                    nc.tensor.matmul(pst[:, j * NMM:(j + 1) * NMM],
                                     t1[:, ms], s1[:, js],
                                     start=False, stop=True)
                nc.scalar.activation(ebuf[:, qs], pst[:], ACTF.Exp,
                                     bias=base_t[:], scale=1.0)
                if q == 1:
                    if pending[0] is not None:
                        pending[0]()
                        pending[0] = None
                    nc.vector.tensor_tensor(gA[:], ebuf[:, 0:QCOL],
                                            ebuf[:, QCOL:2 * QCOL],
                                            op=ALU.max)
                elif q == 2:
                    nc.vector.tensor_tensor(gB[:], gA[:], ebuf[:, qs],
                                            op=ALU.max)
                elif q == 3:
                    nc.vector.tensor_tensor(gC[:], gB[:], ebuf[:, qs],
                                            op=ALU.max)

            nc.vector.tensor_tensor(gA[:, 0:1024], gC[:, 0:1024],
                                    gC[:, 1024:2048], op=ALU.max)
            nc.vector.tensor_tensor(gB[:, 0:512], gA[:, 0:512],
                                    gA[:, 512:1024], op=ALU.max)
            nc.vector.tensor_reduce(mm_t[:], gB[:, 0:512],
                                    axis=mybir.AxisListType.X, op=ALU.max)

            def finish(m=m, ms=ms, last=last, ebuf=ebuf, mm_t=mm_t):
                lnm = stats.tile([P, 1], f32, tag="lnm", name="lnm")
                nc.scalar.activation(lnm[:], mm_t[:], ACTF.Ln)
                rec = stats.tile([P, 1], f32, tag="rec", name="rec")
                nc.vector.reciprocal(rec[:], mm_t[:])
                cvec = stats.tile([P, 1], f32, tag="cvec", name="cvec")
                nc.vector.tensor_tensor(cvec[:], lnm[:], rec[:], op=ALU.mult)
                if last:
                    # tail: 2048 chunks, one on ACT concurrent with DVE,
                    # DMAs issued as each chunk lands
                    for h in range(4):
                        hs = slice(h * QCOL, (h + 1) * QCOL)
                        ob = out_pool.tile([P, QCOL], f16, tag="obl")
                        nc.vector.tensor_scalar(ob[:], ebuf[:, hs],
                                                cvec[:], None, op0=ALU.mult)
                        nc.sync.dma_start(out=out_ext[ms, hs], in_=ob[:])
                else:
                    for h in range(2):
                        hs = slice(h * 4096, (h + 1) * 4096)
                        ob = out_pool.tile([P, 4096], f16, tag="ob")
                        nc.vector.tensor_scalar(ob[:], ebuf[:, hs], cvec[:],
                                                None, op0=ALU.mult)
                        nc.sync.dma_start(out=out_ext[ms, hs], in_=ob[:])

            if last:
                finish()
            else:
                pending[0] = finish

        for m in range(M_TILES):
            run_mtile(m)

    nc.compile()
    return nc


_NC = None


def _get_nc():
    global _NC
    if _NC is None:
        _NC = _build()
    return _NC


def _prep(arr):
    """[N, 64] f32 levels -> [128, N] f16 feature-major, features duplicated."""
    t = np.ascontiguousarray(arr.T.astype(np.float16))       # [64, N]
    return np.ascontiguousarray(np.concatenate([t, t], axis=0))  # [128, N]


def _prep_onehot(arr, lvl0, lvl1):
    """[N, 64] levels -> [128, N] f16: [1[x==lvl0] ; 1[x==lvl1]]."""
    t = arr.T
    return np.ascontiguousarray(
        np.concatenate([(t == lvl0), (t == lvl1)], axis=0).astype(np.float16))


def make_in_maps(bandwidths, test_Xs, train_Xs):
    bandwidths = np.ascontiguousarray(bandwidths, dtype=np.float32)
    test_Xs = np.asarray(test_Xs, dtype=np.float32)
    train_Xs = np.asarray(train_Xs, dtype=np.float32)
    trainA = _prep_onehot(train_Xs, 0.0, 1.0)
    trainB = _prep_onehot(train_Xs, 2.0, 3.0)
    return [
        {
            "bandwidths": bandwidths,
            "testT": _prep(test_Xs[i * M_LOC:(i + 1) * M_LOC]),
            "trainA": trainA,
            "trainB": trainB,
        }
        for i in range(N_CORES)
    ]


def kernel(bandwidths, test_Xs, train_Xs):
    nc = _get_nc()
    in_maps = make_in_maps(bandwidths, test_Xs, train_Xs)
    res = run_bass_kernel_spmd(nc, in_maps, core_ids=list(range(N_CORES)))
    return np.concatenate([np.asarray(r["out"]).astype(np.float32)
                           for r in res.results], axis=0)


if __name__ == "__main__":
    rng = np.random.default_rng(0)
    h = rng.uniform(0.05, 0.5, N_FEAT).astype(np.float32)
    t = rng.integers(0, 4, (N_TEST, N_FEAT)).astype(np.float32)
    s = rng.integers(0, 4, (N_TRAIN, N_FEAT)).astype(np.float32)
    out = kernel(bandwidths=h, test_Xs=t, train_Xs=s)
    print(out.shape, out.dtype)


# revision 30
# speedup vs baseline: 1.0145x; 1.0145x over previous
"""Aitchison-Aitken categorical kernel on 8 TRN2 NeuronCores.

Math (reference, NUM_LEVELS=4, n_feat=64):
    w_f     = log(1-h_f) - log(h_f/3)
    base    = sum_f log(h_f/3) - sum_f log(h_f) = -64*log(3)   (data independent)
    match   = sum_f w_f * 1[test_if == train_jf]           ([n_test, n_train])
    ld      = match + base
    out     = rowmax(ld) * exp(ld - rowmax(ld))
          ( = c * exp(ld) with c = ln(m)/m, m = e^{rowmax ld} )

Device algorithm (per core, data-parallel over test rows):
  - host pre-lays-out test/train as feature-major f16 with the 64 features
    duplicated to 128 partitions ([dup | dup]); the device then needs no
    transposes, no PSUM encode staging, and only ~300 DMA descriptors
  - level-indicator encodings A=[lvl0|lvl1], B=[lvl2|lvl3] via DVE
    tensor_scalar is_equal (w folded into the test side), K = 2 x 128
  - per m-tile (128 test rows): 4 psum blocks of [128,2048] f32 (4 banks,
    ring of 2), each = 4 matmuls of 1024 cols (A,A,B,B accumulate)
  - exp(match + base) via one ScalarE activation per 2048 block,
    PSUM -> bf16 ebuf (bf16: e^ld spans e^-157..e^+41)
  - row max via a running DVE tensor_tensor max fold (2x bf16), folded
    2048 -> 512 before the single 1x tensor_reduce
  - c = ln(m)/m; obuf = ebuf * c (DVE tensor_scalar 4x); DMA per 4096 cols
  - the finish chain (ln, cvec, mults, DMAs) of m-tile m is emitted
    between m+1's exps to keep the ACT queue bubble-free

Sharding: test_Xs rows across 8 cores; bandwidths/train_Xs replicated;
out [1024, 8192] f16 local per core, host-concatenated.
"""
import numpy as np
from contextlib import ExitStack

from concourse import bacc, hw_specs, mybir, tile
from concourse.bass_utils import run_bass_kernel_spmd

f32 = mybir.dt.float32
f16 = mybir.dt.float16
bf16 = mybir.dt.bfloat16
ACTF = mybir.ActivationFunctionType
ALU = mybir.AluOpType

N_CORES = 8
N_TEST, N_TRAIN, N_FEAT = 8192, 8192, 64
M_LOC = N_TEST // N_CORES          # 1024 test rows per core
P = 128                            # partitions
M_TILES = M_LOC // P               # 8
QCOL = 2048                        # psum block (4 banks f32)
NQ = 4                             # blocks per m-tile
NMM = 512                          # cols per matmul (one PSUM bank)
BASE = float(-N_FEAT * np.log(3.0))
NEG_INF = float(np.float32(-3.0e38))


# ---------------------------------------------------------------------------
# Activation-table patch: both Exp and Ln are needed; the stock fixpoint pass
# resolves each to the first table set containing it (exp_and_others vs
# natural_log), reloading the ACT tables (~1.3us) on every switch. Hide
# Exp/Ln from every other set so both resolve to natural_log_exp_and_others
# at its true act_func_set_id (list order/indices preserved).
_COMBINED = "natural_log_exp_and_others"
_orig_get_tables = hw_specs.get_activation_tables


def _patched_tables(module_arch):
    d = _orig_get_tables(module_arch)
    if _COMBINED not in d:
        return d
    hide = {ACTF.Exp, ACTF.Ln}
    return {k: (set(v) if k == _COMBINED else set(v) - hide)
            for k, v in d.items()}


hw_specs.get_activation_tables = _patched_tables
bacc.get_activation_tables = _patched_tables   # bacc imported its own ref
# ---------------------------------------------------------------------------


def _build():
    nc = bacc.Bacc(None, target_bir_lowering=False)
    bw_ext = nc.declare_dram_parameter("bandwidths", [N_FEAT], f32, isOutput=False)
    # host-prepped: feature-major f16, features duplicated across halves
    test_ext = nc.declare_dram_parameter("testT", [P, M_LOC], f16, isOutput=False)
    train_ext = nc.declare_dram_parameter("trainT", [P, N_TRAIN], f16, isOutput=False)
    out_ext = nc.declare_dram_parameter("out", [M_LOC, N_TRAIN], f16, isOutput=True)

    with tile.TileContext(nc) as tc, ExitStack() as ctx:
        const = ctx.enter_context(tc.tile_pool(name="const", bufs=1))
        ps = ctx.enter_context(tc.tile_pool(name="ps", bufs=2, space="PSUM"))
        enc = ctx.enter_context(tc.tile_pool(name="enc", bufs=1))
        stats = ctx.enter_context(tc.tile_pool(name="stats", bufs=4))
        scr = ctx.enter_context(tc.tile_pool(name="scr", bufs=3))
        ebuf_pool = ctx.enter_context(tc.tile_pool(name="ebuf", bufs=3))
        out_pool = ctx.enter_context(tc.tile_pool(name="obuf", bufs=4))

        # ---- input DMAs first, all on the sync HWDGE (RTL descriptor gen;
        # the gpsimd SWDGE takes ~6us to software-generate 128 descriptors).
        # Chunk 0 and bandwidths lead since they gate the first matmul;
        # each train chunk is its own tile so its is_equal only waits for
        # its own DMA. ----------------------------------------------------
        NCH = 4
        CH = N_TRAIN // NCH
        dstT_s = []
        st0 = enc.tile([P, CH], f16, name="dstT_s0")
        nc.sync.dma_start(out=st0[:], in_=train_ext[:, 0:CH])
        dstT_s.append(st0)
        bw = const.tile([N_FEAT, 1], f32)
        nc.sync.dma_start(out=bw[:], in_=bw_ext[:].rearrange("(f o) -> f o", o=1))
        dstT_t = enc.tile([P, M_LOC], f16)
        nc.sync.dma_start(out=dstT_t[:], in_=test_ext[:])
        for c in range(1, NCH):
            st = enc.tile([P, CH], f16, name=f"dstT_s{c}")
            nc.sync.dma_start(out=st[:], in_=train_ext[:, c * CH:(c + 1) * CH])
            dstT_s.append(st)

        one_t = const.tile([N_FEAT, 1], f32)
        nc.vector.memset(one_t[:], 1.0)
        base_t = const.tile([P, 1], f32)
        nc.vector.memset(base_t[:], BASE)
        # dummy activation: triggers the combined Exp/Ln table load now
        scratch = const.tile([P, 1], f32)
        nc.scalar.activation(scratch[:], base_t[:], ACTF.Exp)
        lvlA = const.tile([P, 1], f32)
        nc.vector.memset(lvlA[0:64, :], 0.0)
        nc.vector.memset(lvlA[64:128, :], 1.0)
        lvlB = const.tile([P, 1], f32)
        nc.vector.memset(lvlB[0:64, :], 2.0)
        nc.vector.memset(lvlB[64:128, :], 3.0)

        # ---- w vector from bandwidths ----------------------------------
        lt = const.tile([N_FEAT, 1], f32)   # log(1 - h)
        nc.scalar.activation(lt[:], bw[:], ACTF.Ln, bias=one_t[:], scale=-1.0)
        lf = const.tile([N_FEAT, 1], f32)   # log(h/3)
        nc.scalar.activation(lf[:], bw[:], ACTF.Ln, scale=1.0 / 3.0)

        # ---- encodings: is_equal (DVE), w folded into test side.
        # Emission order tuned for the lead-in: train chunk0 A first (its
        # DMA lands before bw), then the w chain + test encodings (gating
        # the first matmuls), then the rest.
        tencA = enc.tile([P, M_LOC], f16)
        tencB = enc.tile([P, M_LOC], f16)
        sencA = enc.tile([P, N_TRAIN], f16)
        sencB = enc.tile([P, N_TRAIN], f16)

        def senc_chunk(c, which):
            cs = slice(c * CH, (c + 1) * CH)
            dst, lvl = (sencA, lvlA) if which == 0 else (sencB, lvlB)
            nc.vector.tensor_scalar(dst[:, cs], dstT_s[c][:], lvl[:], None,
                                    op0=ALU.is_equal)

        w2 = const.tile([P, 1], f32)
        nc.vector.tensor_tensor(w2[0:64, :], lt[:], lf[:], op=ALU.subtract)
        nc.vector.tensor_copy(w2[64:128, :], w2[0:64, :])
        senc_chunk(0, 0)
        nc.vector.tensor_scalar(tencA[:], dstT_t[:], lvlA[:], w2[:],
                                op0=ALU.is_equal, op1=ALU.mult)
        senc_chunk(0, 1)
        nc.vector.tensor_scalar(tencB[:], dstT_t[:], lvlB[:], w2[:],
                                op0=ALU.is_equal, op1=ALU.mult)
        for c in range(1, NCH):
            senc_chunk(c, 0)
            senc_chunk(c, 1)

        # ---- PE warm-up: the HAM clock gate starts matmuls at ~half rate
        # until ~4-5us of sustained activity; burn dummy matmuls into the
        # first psum tile while the input DMAs land so the real first
        # m-tile runs at full clock (start=True re-clears has_written).
        warm = const.tile([P, NMM], f16)
        nc.vector.memset(warm[:], 0.0)
        warm_pst = ps.tile([P, QCOL], f32, tag="mm")
        for i in range(10):
            nc.tensor.matmul(warm_pst[:, (i % 4) * NMM:(i % 4 + 1) * NMM],
                             warm[:, 0:P], warm[:], start=True, stop=True)

        # ---- main loop --------------------------------------------------
        # The finish chain of m-tile m (ln -> recip -> cvec -> mults ->
        # DMAs) is deferred and emitted between m+1's exp q1 and q2.  By
        # then m's DVE max-reduce is long done, so ACT reaches the Ln with
        # no wait and the queue never bubbles; m's output mults run on DVE
        # while ACT exps m+1's later blocks.
        pending = [None]

        def run_mtile(m):
            ms = slice(m * P, (m + 1) * P)
            last = m == M_TILES - 1
            ebuf = ebuf_pool.tile([P, N_TRAIN], bf16)
            mm_t = stats.tile([P, 1], f32, tag="mm_t", name="mm_t")
            # running pairwise-max fold over ebuf: gA = eb0 v eb1 (after
            # q1, runs during q2's exps), gB = gA v eb2, gC = gB v eb3,
            # then fold 2048 -> 512 and one 1x reduce (single-operand
            # reduce is 1x-only on DVE: keep its input short).
            gA = scr.tile([P, QCOL], bf16, tag="sc")
            gB = scr.tile([P, QCOL], bf16, tag="sc")
            gC = scr.tile([P, QCOL], bf16, tag="sc")
            if last and pending[0] is not None:
                # flush m-2's finish now so the DVE queue is clear for
                # this tile's fold ladder (its tail is the kernel tail);
                # costs at most a short ACT wait on ln once
                pending[0]()
                pending[0] = None

            for q in range(NQ):
                qs = slice(q * QCOL, (q + 1) * QCOL)
                pst = ps.tile([P, QCOL], f32, tag="mm")
                # A,A,B,B per block; alternate order across blocks so the
                # boundary stationary is shared (ldweights pull-ahead hides
                # the rest)
                order = ((tencA, sencA), (tencB, sencB))
                if q % 2 == 1:
                    order = (order[1], order[0])
                (t0, s0), (t1, s1) = order
                for j in range(QCOL // NMM):
                    js = slice(q * QCOL + j * NMM, q * QCOL + (j + 1) * NMM)
                    nc.tensor.matmul(pst[:, j * NMM:(j + 1) * NMM],
                                     t0[:, ms], s0[:, js],
                                     start=True, stop=False)
                for j in range(QCOL // NMM):
                    js = slice(q * QCOL + j * NMM, q * QCOL + (j + 1) * NMM)
                    nc.tensor.matmul(pst[:, j * NMM:(j + 1) * NMM],
                                     t1[:, ms], s1[:, js],
                                     start=False, stop=True)
                nc.scalar.activation(ebuf[:, qs], pst[:], ACTF.Exp,
                                     bias=base_t[:], scale=1.0)
                if q == 1:
                    if pending[0] is not None:
                        pending[0]()
                        pending[0] = None
                    nc.vector.tensor_tensor(gA[:], ebuf[:, 0:QCOL],
                                            ebuf[:, QCOL:2 * QCOL],
                                            op=ALU.max)
                elif q == 2:
                    nc.vector.tensor_tensor(gB[:], gA[:], ebuf[:, qs],
                                            op=ALU.max)
                elif q == 3:
                    nc.vector.tensor_tensor(gC[:], gB[:], ebuf[:, qs],
                                            op=ALU.max)

            nc.vector.tensor_tensor(gA[:, 0:1024], gC[:, 0:1024],
                                    gC[:, 1024:2048], op=ALU.max)
            nc.vector.tensor_tensor(gB[:, 0:512], gA[:, 0:512],
                                    gA[:, 512:1024], op=ALU.max)
            nc.vector.tensor_reduce(mm_t[:], gB[:, 0:512],
                                    axis=mybir.AxisListType.X, op=ALU.max)

            def finish(m=m, ms=ms, last=last, ebuf=ebuf, mm_t=mm_t):
                lnm = stats.tile([P, 1], f32, tag="lnm", name="lnm")
                nc.scalar.activation(lnm[:], mm_t[:], ACTF.Ln)
                rec = stats.tile([P, 1], f32, tag="rec", name="rec")
                nc.vector.reciprocal(rec[:], mm_t[:])
                cvec = stats.tile([P, 1], f32, tag="cvec", name="cvec")
                nc.vector.tensor_tensor(cvec[:], lnm[:], rec[:], op=ALU.mult)
                if last:
                    # tail: 2048 chunks, one on ACT concurrent with DVE,
                    # DMAs issued as each chunk lands
                    for h in range(4):
                        hs = slice(h * QCOL, (h + 1) * QCOL)
                        ob = out_pool.tile([P, QCOL], f16, tag="obl")
                        nc.vector.tensor_scalar(ob[:], ebuf[:, hs],
                                                cvec[:], None, op0=ALU.mult)
                        nc.sync.dma_start(out=out_ext[ms, hs], in_=ob[:])
                else:
                    for h in range(2):
                        hs = slice(h * 4096, (h + 1) * 4096)
                        ob = out_pool.tile([P, 4096], f16, tag="ob")
                        nc.vector.tensor_scalar(ob[:], ebuf[:, hs], cvec[:],
                                                None, op0=ALU.mult)
                        nc.sync.dma_start(out=out_ext[ms, hs], in_=ob[:])

            if last:
                finish()
            else:
                pending[0] = finish

        for m in range(M_TILES):
            run_mtile(m)

    nc.compile()
    return nc


_NC = None


def _get_nc():
    global _NC
    if _NC is None:
        _NC = _build()
    return _NC


def _prep(arr):
    """[N, 64] f32 levels -> [128, N] f16 feature-major, features duplicated."""
    t = np.ascontiguousarray(arr.T.astype(np.float16))       # [64, N]
    return np.ascontiguousarray(np.concatenate([t, t], axis=0))  # [128, N]


def make_in_maps(bandwidths, test_Xs, train_Xs):
    bandwidths = np.ascontiguousarray(bandwidths, dtype=np.float32)
    test_Xs = np.asarray(test_Xs, dtype=np.float32)
    train_Xs = np.asarray(train_Xs, dtype=np.float32)
    trainT = _prep(train_Xs)
    return [
        {
            "bandwidths": bandwidths,
            "testT": _prep(test_Xs[i * M_LOC:(i + 1) * M_LOC]),
            "trainT": trainT,
        }
        for i in range(N_CORES)
    ]


def kernel(bandwidths, test_Xs, train_Xs):
    nc = _get_nc()
    in_maps = make_in_maps(bandwidths, test_Xs, train_Xs)
    res = run_bass_kernel_spmd(nc, in_maps, core_ids=list(range(N_CORES)))
    return np.concatenate([np.asarray(r["out"]).astype(np.float32)
                           for r in res.results], axis=0)


if __name__ == "__main__":
    rng = np.random.default_rng(0)
    h = rng.uniform(0.05, 0.5, N_FEAT).astype(np.float32)
    t = rng.integers(0, 4, (N_TEST, N_FEAT)).astype(np.float32)
    s = rng.integers(0, 4, (N_TRAIN, N_FEAT)).astype(np.float32)
    out = kernel(bandwidths=h, test_Xs=t, train_Xs=s)
    print(out.shape, out.dtype)
